# revision 1
# baseline (speedup 1.0000x reference)
"""Trainium2 Bass kernel for causal multi-head attention with ALiBi.

Computes, for x:[B,S,D]:
    qkv = x @ W_packed.T + b_packed ; q,k,v = split(qkv)
    heads -> scores = q k^T / sqrt(hd) + alibi_causal_bias
    out = softmax(scores) v -> merge heads -> out @ W_out.T + b_out

Sharding (8 cores): core c handles batch c//4 and heads {k, k+4, k+8, k+12}
(k = c%4), one head per "slot". Slot block-schedules are head-independent
(sized for the largest ALiBi window in the slot), so one SPMD program runs
on all 8 cores; only the data (weight slices, bias tables) differs.
Host sums the 4 out-projection partials per batch and adds
b_out + W_out @ b_v (the v-bias term commutes through attention).

ALiBi sparsity: head h attends effectively only ~35/slope_h positions back;
beyond that its contribution is < e^-23 relative. Slots keep only the
causal k-blocks within that window (KEEP blocks).

Softmax without row-max: scores are O(+-6), and exp is recentred per
q-group by C_g (a per-column constant that cancels in normalization),
keeping exp args in fp32 range. In the transposed layout scoresT[k,q] the
recentred ALiBi bias slope*(k - C_g) is per-partition, so it rides the
single Exp activation for free. Row sums come from a ones-row appended to
v in the attn@v matmul; normalization divides by that row.
"""

import os
import sys

import numpy as np

for _p in ("/opt/trn_rl_repo",):
    if os.path.isdir(_p) and _p not in sys.path:
        sys.path.append(_p)

import concourse.bacc as bacc
import concourse.bass as bass
import concourse.tile as tile
from concourse import mybir
from concourse.bass_utils import run_bass_kernel_spmd

B, S, D, H, HD = 2, 2048, 1024, 16, 64
NBLK = S // 128          # 16 k/q blocks
NCORES = 8

F32 = mybir.dt.float32
F32R = mybir.dt.float32r
BF16 = mybir.dt.bfloat16

# Slots A..D: per-core heads [12+k, 8+k, 4+k, k].  KEEP = causal k-blocks
# kept per q-block (window d_h = 35/slope_h, slot max).  W = q-group width.
SLOT_KEEP = (17, 17, 6, 3)
SLOT_W = (512, 512, 512, 128)
SLOT_OFF0 = (128, 128, 128, 64)
SLOT_TABW = tuple(k + 3 if w == 512 else k for k, w in zip(SLOT_KEEP, SLOT_W))
SLOT_TABOFF = tuple(int(np.cumsum((0,) + SLOT_TABW)[i]) for i in range(4))
TABW = int(sum(SLOT_TABW))  # 52


def _slot_schedule(s):
    """Yield (g, q0, W, [(j, lo, hi, tabcol, isdiag), ...]) per q-group."""
    K, W, _ = SLOT_KEEP[s], SLOT_W[s], SLOT_OFF0[s]
    out = []
    if W == 512:
        for g in range(S // 512):
            jlo = max(0, 4 * g + 3 - (K - 1))
            blocks = []
            for j in range(jlo, 4 * g + 4):
                lo = max(0, (j - 4 * g) * 128)
                hi = min(512, (j - 4 * g + K) * 128)
                m = j - 4 * g + (K - 1)
                blocks.append((j, lo, hi, SLOT_TABOFF[s] + m, j >= 4 * g))
            out.append((g, g * 512, 512, blocks))
    else:
        for i in range(NBLK):
            blocks = []
            for j in range(max(0, i - (K - 1)), i + 1):
                m = j - i + (K - 1)
                blocks.append((j, 0, 128, SLOT_TABOFF[s] + m, j == i))
            out.append((i, i * 128, 128, blocks))
    return out


def build_program():
    nc = bacc.Bacc("TRN2", target_bir_lowering=False, debug=False,
                   num_devices=NCORES)

    xT = nc.dram_tensor("xT", [D, S], BF16, kind="ExternalInput")
    wqkT = nc.dram_tensor("wqkT", [D, 512], BF16, kind="ExternalInput")
    wvT = nc.dram_tensor("wvT", [D, 256], BF16, kind="ExternalInput")
    woT = nc.dram_tensor("woT", [256, D], BF16, kind="ExternalInput")
    bqk = nc.dram_tensor("bqk", [128, 4], F32, kind="ExternalInput")
    btab = nc.dram_tensor("btab", [128, TABW], F32, kind="ExternalInput")
    onesd = nc.dram_tensor("onesd", [65, 64], F32R, kind="ExternalInput")
    out = nc.dram_tensor("out", [S, D], BF16, kind="ExternalOutput")

    with tile.TileContext(nc) as tc:
        with tc.tile_pool(name="persist", bufs=1) as pp:
            qkT = [pp.tile([128, S], BF16, tag=f"qkT{t}", name=f"qkT{t}")
                   for t in range(4)]
            v_t = pp.tile([128, 4, NBLK, 65], BF16, tag="v", name="v")
            hoT = [pp.tile([128, S], BF16, tag=f"hoT{t}", name=f"hoT{t}")
                   for t in range(2)]
            btab_sb = pp.tile([128, TABW], F32, tag="btab", name="btab")
            bqk_sb = pp.tile([128, 4], F32, tag="bqk", name="bqk")
            ones_r = pp.tile([65, 64], F32R, tag="ones_r", name="ones_r")

            nc.sync.dma_start(btab_sb[:], btab[:])
            nc.sync.dma_start(bqk_sb[:], bqk[:])
            nc.gpsimd.memset(v_t[:, :, :, 64:65], 1.0)
            nc.sync.dma_start(ones_r[:], onesd[:])

            wo_sb = []
            for cc in range(2):
                t = pp.tile([128, D], BF16, tag=f"wo{cc}", name=f"wo{cc}")
                nc.sync.dma_start(t[:], woT[cc * 128:(cc + 1) * 128, :])
                wo_sb.append(t)

            # PSUM: 8 banks as 4 tags; phase-1 QKV borrows all four tags
            with (
                tc.tile_pool(name="xw", bufs=1) as xw,
                tc.tile_pool(name="et", bufs=8) as etp,
                tc.tile_pool(name="nrm", bufs=3) as nrm,
                tc.tile_pool(name="ob", bufs=2) as obp,
                tc.tile_pool(name="ps_sc", bufs=3, space="PSUM") as sc_ps,
                tc.tile_pool(name="ps_av", bufs=2, space="PSUM") as av_ps,
                tc.tile_pool(name="ps_bp", bufs=1, space="PSUM") as bp_ps,
                tc.tile_pool(name="ps_op", bufs=2, space="PSUM") as op_ps,
            ):
                xT_sb, wqk_sb, wv_sb = [], [], []
                for m in range(8):
                    t = xw.tile([128, 512], BF16, tag=f"wqk{m}",
                                name=f"wqk{m}")
                    nc.sync.dma_start(t[:], wqkT[m * 128:(m + 1) * 128, :])
                    wqk_sb.append(t)
                    t = xw.tile([128, S], BF16, tag=f"x{m}", name=f"x{m}")
                    nc.sync.dma_start(t[:], xT[m * 128:(m + 1) * 128, :])
                    xT_sb.append(t)
                    t = xw.tile([128, 256], BF16, tag=f"wv{m}", name=f"wv{m}")
                    nc.sync.dma_start(t[:], wvT[m * 128:(m + 1) * 128, :])
                    wv_sb.append(t)

                def p1_tile(i, w):
                    pool, tag = [(sc_ps, "sc"), (sc_ps, "sc"), (sc_ps, "sc"),
                                 (bp_ps, "bps"), (av_ps, "av"), (av_ps, "av"),
                                 (op_ps, "op"), (op_ps, "op")][i]
                    return pool.tile([128, w], F32, tag=tag, name=f"p1_{i}")

                def qk_half(half):
                    # quarters {2h,2h+1} x 4 f-tiles -> 8 one-bank psums
                    # (m-outer: first matmul waits only for chunk-0 DMAs)
                    pss = {}
                    for ft in range(4):
                        for qi in range(2):
                            pss[ft, qi] = p1_tile(ft * 2 + qi, 512)
                    for m in range(8):
                        for ft in range(4):
                            for qi in range(2):
                                q4 = half * 2 + qi
                                nc.tensor.matmul(
                                    pss[ft, qi][:],
                                    wqk_sb[m][:, ft * 128:(ft + 1) * 128],
                                    xT_sb[m][:, q4 * 512:(q4 + 1) * 512],
                                    start=(m == 0), stop=(m == 7),
                                )
                    for ft in range(4):
                        for qi in range(2):
                            q4 = half * 2 + qi
                            scol = slice(q4 * 512, (q4 + 1) * 512)
                            # psum*scale + bias (1/sqrt(hd) folded into q)
                            nc.vector.tensor_scalar(
                                out=qkT[ft][:, scol], in0=pss[ft, qi][:],
                                scalar1=(0.125 if ft < 2 else 1.0),
                                scalar2=bqk_sb[:, ft:ft + 1],
                                op0=mybir.AluOpType.mult,
                                op1=mybir.AluOpType.add,
                            )

                def v_half(half):
                    # 256-wide v accumulation groups: one psum bank each
                    # (zero-regions are bank-granular)
                    pss = {sbi: p1_tile(sbi, 256) for sbi in range(8)}
                    for m in range(8):
                        for sbi in range(8):
                            sb = half * 8 + sbi
                            nc.tensor.matmul(
                                pss[sbi][:],
                                xT_sb[m][:, sb * 128:(sb + 1) * 128],
                                wv_sb[m][:],
                                start=(m == 0), stop=(m == 7),
                            )
                    for sbi in range(8):
                        sb = half * 8 + sbi
                        nc.vector.tensor_copy(
                            v_t[:, :, sb, 0:64],
                            pss[sbi][:].rearrange("p (s c) -> p s c", s=4),
                        )

                qk_half(0)
                v_half(0)
                qk_half(1)
                v_half(1)

                # ---- attention (+ out-proj interleaved per q-group) ----
                sched = [_slot_schedule(s) for s in range(4)]

                def scores_av(s, ent, av, coff):
                    """Scores+exp+AV for one q-group into av[:, coff:+W]."""
                    po = (s % 2) * 64
                    qT_s = qkT[s // 2][po:po + 64, :]
                    kT_s = qkT[2 + s // 2][po:po + 64, :]
                    g, q0, W, blocks = ent
                    for bi, (j, lo, hi, tcol, isdiag) in enumerate(blocks):
                        sc = sc_ps.tile([128, W], F32, tag="sc", name="sc")
                        nc.tensor.matmul(
                            sc[:],
                            kT_s[:, j * 128:(j + 1) * 128],
                            qT_s[:, q0:q0 + W],
                        )
                        et = etp.tile([128, W], BF16, tag="et", name="et")
                        if lo > 0 or hi < W:
                            nc.gpsimd.memset(et[:], 0.0)
                        nc.scalar.activation(
                            et[:, lo:hi], sc[:, lo:hi],
                            mybir.ActivationFunctionType.Exp,
                            bias=btab_sb[:, tcol:tcol + 1], scale=1.0,
                        )
                        if isdiag:
                            # zero k>q inside the diagonal 128x128 block
                            nc.gpsimd.affine_select(
                                out=et[:, lo:lo + 128],
                                in_=et[:, lo:lo + 128],
                                compare_op=mybir.AluOpType.is_ge,
                                fill=0.0, base=0,
                                pattern=[[1, 128]],
                                channel_multiplier=-1,
                            )
                        nc.tensor.matmul(
                            av[:, coff:coff + W], v_t[:, s, j, :], et[:],
                            start=(bi == 0), stop=(bi == len(blocks) - 1),
                        )

                def norm(s, av, q0, W):
                    """Divide av[0:64] by the ones-row sum; write hoT."""
                    po = (s % 2) * 64
                    hoT_s = hoT[s // 2]
                    lr = nrm.tile([65, W], F32R, tag="lr", name="lr")
                    nc.vector.tensor_copy(lr[64:65, :], av[64:65, :])
                    bps = bp_ps.tile([64, W], F32, tag="bps", name="bps")
                    nc.tensor.matmul(
                        bps[:], ones_r[64:65, 0:64], lr[64:65, :])
                    binv = nrm.tile([64, W], F32, tag="binv", name="binv")
                    nc.vector.reciprocal_approx_fast(out=binv[:], in_=bps[:])
                    if po == 0:
                        nc.vector.tensor_mul(
                            hoT_s[0:64, q0:q0 + W], av[0:64, :], binv[:])
                    else:
                        # DVE lanes can't shift partitions; bounce via DMA
                        tmp = nrm.tile([64, W], BF16, tag="hotmp",
                                       name="hotmp")
                        nc.vector.tensor_mul(tmp[:], av[0:64, :], binv[:])
                        nc.gpsimd.dma_start(
                            hoT_s[64:128, q0:q0 + W], tmp[:])

                def attn_group(s, ent):
                    g, q0, W, blocks = ent
                    av = av_ps.tile([65, W], F32, tag="av", name="av")
                    scores_av(s, ent, av, 0)
                    norm(s, av, q0, W)

                def op_block(sb):
                    ob = obp.tile([128, D], BF16, tag="ob", name="ob")
                    for jh in range(2):
                        ps = op_ps.tile([128, 512], F32, tag="op", name="op")
                        for cc in range(2):
                            nc.tensor.matmul(
                                ps[:],
                                hoT[cc][:, sb * 128:(sb + 1) * 128],
                                wo_sb[cc][:, jh * 512:(jh + 1) * 512],
                                start=(cc == 0), stop=(cc == 1),
                            )
                        nc.vector.tensor_copy(
                            ob[:, jh * 512:(jh + 1) * 512], ps[:])
                    nc.gpsimd.dma_start(out[sb * 128:(sb + 1) * 128, :],
                                        ob[:])

                for g in range(4):
                    for s in range(3):
                        attn_group(s, sched[s][g])
                    for i4 in range(4):
                        attn_group(3, sched[3][4 * g + i4])
                    for sb in range(4 * g, 4 * g + 4):
                        op_block(sb)

    nc.compile()
    return nc


def make_core_inputs(c, x, W_packed, b_packed):
    """Host-side shard prep for core c (pure numpy reshuffles)."""
    k, b = c % 4, c // 4
    heads = [12 + k, 8 + k, 4 + k, k]          # slots A..D
    rows = np.concatenate([np.arange(h * 64, (h + 1) * 64) for h in heads])

    xT = np.ascontiguousarray(x[b].T)                       # [D, S]
    wq = W_packed[rows]                                     # [256, D]
    wk = W_packed[D + rows]
    wv = W_packed[2 * D + rows]
    wqkT = np.ascontiguousarray(np.concatenate([wq, wk], 0).T)  # [D, 512]
    wvT = np.ascontiguousarray(wv.T)                        # [D, 256]
    woT = None  # filled by caller (needs W_out)

    bq = b_packed[rows] / 8.0
    bk = b_packed[D + rows]
    bqk = np.stack([bq[:128], bq[128:], bk[:128], bk[128:]], 1)  # [128, 4]
    bqk = np.ascontiguousarray(bqk, dtype=np.float32)

    btab = np.zeros((128, TABW), np.float32)
    p = np.arange(128, dtype=np.float64)[:, None]
    for s in range(4):
        h = heads[s]
        slope = 2.0 ** (-(h + 1) * 8.0 / H)
        K, off0, tw, to = SLOT_KEEP[s], SLOT_OFF0[s], SLOT_TABW[s], SLOT_TABOFF[s]
        m = np.arange(tw, dtype=np.float64)[None, :]
        btab[:, to:to + tw] = (slope * (p + 128.0 * (m - (K - 1)) - off0)
                               ).astype(np.float32)
    import ml_dtypes
    ones = np.ones((65, 64), np.float32)
    return heads, {"xT": xT.astype(ml_dtypes.bfloat16),
                   "wqkT": wqkT.astype(ml_dtypes.bfloat16),
                   "wvT": wvT.astype(ml_dtypes.bfloat16),
                   "bqk": bqk, "btab": btab, "onesd": ones}


_NC_CACHE = {}


def _get_program():
    if "nc" not in _NC_CACHE:
        _NC_CACHE["nc"] = build_program()
    return _NC_CACHE["nc"]


def kernel(x, W_packed, b_packed, W_out, b_out):
    x = np.asarray(x, np.float32)
    W_packed = np.asarray(W_packed, np.float32)
    b_packed = np.asarray(b_packed, np.float32)
    W_out = np.asarray(W_out, np.float32)
    b_out = np.asarray(b_out, np.float32)

    nc = _get_program()

    in_maps = []
    for c in range(NCORES):
        heads, m = make_core_inputs(c, x, W_packed, b_packed)
        cols = np.concatenate([np.arange(h * 64, (h + 1) * 64) for h in heads])
        import ml_dtypes
        m["woT"] = np.ascontiguousarray(W_out[:, cols].T).astype(
            ml_dtypes.bfloat16)
        in_maps.append(m)

    res = run_bass_kernel_spmd(nc, in_maps, core_ids=list(range(NCORES)))

    # Gather: sum partials per batch; add b_out and the folded v-bias term.
    b_v = b_packed[2 * D:]
    bias_row = (b_out + W_out @ b_v).astype(np.float32)     # [D]
    full = np.empty((B, S, D), np.float32)
    for b in range(B):
        acc = res.results[4 * b]["out"].astype(np.float32).copy()
        for c in range(4 * b + 1, 4 * b + 4):
            acc += res.results[c]["out"]
        full[b] = acc + bias_row
    return full

